# revision 6
# baseline (speedup 1.0000x reference)
"""Euclidean distance-matrix kernel (retrieval kNN) for Trainium2, 8 cores.

out[g, b, k] = || x[g, b, :] - centroids[g, k, :] ||_2
  via d2 = c2[k] - 2 x.c  (matmul, fp16)  then  sqrt(d2 + x2[b])  (ACT,
  x2 added exactly in fp32 through the activation bias).

Sharding: B axis (8192) split across 8 cores (1024 rows each); centroids
replicated; no collectives.

Per-core design (G=16, Bs=1024, K=1024, D=64), memory-roofline targeted:

  - Output is stored fp16 on device (halves the dominant HBM write:
    64 MiB -> 32 MiB per core) and upcast to fp32 on the host. fp16
    rounding of sqrt values adds ~5e-4 rel err, far inside the 2e-2 gate.
  - Both x[g] and c[g] load with b/k = p*8+e chunking so every partition
    reads one contiguous 2KB line (fat descriptors; element-strided DMA
    is ~100x slower). The resulting k-permutation on the centroid side is
    undone for free in the PSUM->SBUF copy after the PE transpose (the
    copy scatters columns with stride 8), so the output's K axis stays
    contiguous for the store.
  - fp16 aug tiles [128, 65] are built on DVE: x cols [x(64); 1],
    c cols [-2c(64); c2]; x2/c2 are exact fp32 square+reduce, fp16-
    rounded only where they ride the matmul (c2) - x2 stays fp32 via the
    ACT bias.
  - 8 PE transposes per operand tile -> [65, 1024] fp16; one 65-row fp16
    matmul per [128 x 512] output block accumulates d2 into PSUM (512 is
    the ISA's moving-operand limit - 1024-wide matmuls fail the walrus
    ISA check).
  - ACT does ONLY the sqrt: one op per [128, 1024] 2-bank PSUM tile
    (3-deep rotation; wider ACT needs 4-bank tiles whose 2-slot rotation
    starves the pipeline - measured slower on HW), writing fp16 straight
    into the store tile.
  - Each group's 2 MiB output leaves SBUF in two DMAs with
    16KB/partition contiguous lines (split halves drain earlier).
  - repeat > 1 runs the body inside a hardware For_i loop (two bodies
    per back-edge when repeat is even), so the NEFF size is constant in
    `repeat` and a large repeat count gives a noise-immune per-iteration
    timing delta: host wall-clock noise here is seconds, ~10000x the
    kernel time. All DRAM loads stay inside the loop (each iteration
    does the full job).
"""

import sys

sys.path.insert(0, "/opt/trn_rl_repo")

import numpy as np

import concourse.bacc as bacc
import concourse.tile as tile
from concourse import mybir
from concourse.bass_utils import run_bass_kernel_spmd
from concourse.masks import make_identity

G, B, K, D = 16, 8192, 1024, 64
N_CORES = 8
BS = B // N_CORES          # 1024 rows per core
E = BS // 128              # 8 row-chunks per group

F32 = mybir.dt.float32
F16 = mybir.dt.float16
SQRT = mybir.ActivationFunctionType.Sqrt

_cache = {}


def build_nc(repeat: int = 1):
    nc = bacc.Bacc("TRN2", target_bir_lowering=False, debug=False,
                   num_devices=N_CORES)
    x_in = nc.dram_tensor("x", [G, BS, D], F32, kind="ExternalInput").ap()
    c_in = nc.dram_tensor("centroids", [G, K, D], F32, kind="ExternalInput").ap()
    out = nc.dram_tensor("out", [G, BS, K], F16, kind="ExternalOutput").ap()
    dma = nc.sync.dma_start

    with tile.TileContext(nc) as tc:
        with (
            tc.tile_pool(name="res", bufs=1) as res,
            tc.tile_pool(name="ld", bufs=3) as ld,
            tc.tile_pool(name="op", bufs=3) as opp,
            tc.tile_pool(name="ob", bufs=3) as obp,
            tc.tile_pool(name="ps", bufs=3, space="PSUM") as ps,
            tc.tile_pool(name="pt", bufs=2, space="PSUM") as pt,
        ):
            ident = res.tile([128, 128], F16, tag="ident")
            make_identity(nc, ident)

            def group_body(g):
                # ---- c[g]: aug rows [-2c(64); c2], k = p*8+e chunking
                ca = ld.tile([128, E, D], F32, tag="ca")
                dma(out=ca, in_=c_in[g].rearrange("(p e) d -> p e d", p=128))
                csq = ld.tile([128, E, D], F32, tag="csq")
                nc.vector.tensor_mul(csq, ca, ca)
                c2 = ld.tile([128, E], F32, tag="c2")
                nc.vector.reduce_sum(c2, csq, axis=mybir.AxisListType.X)
                cf = opp.tile([128, E, 65], F16, tag="cf")
                nc.vector.tensor_scalar_mul(cf[:, :, 0:64], ca, -2.0)
                nc.vector.tensor_copy(cf[:, :, 64], c2)
                psc = pt.tile([65, E, 128], F16, tag="psT")
                for e in range(E):
                    nc.tensor.transpose(psc[:, e], cf[:, e], ident)
                # cT[r, p, e] = psc[r, e, p] -> columns land at k = p*8+e,
                # i.e. cT viewed [65, 1024] is in NATURAL k order.
                cT = opp.tile([65, 128, E], F16, tag="cT")
                nc.vector.tensor_copy(cT.rearrange("r p e -> r e p"), psc)
                cTf = cT.rearrange("r p e -> r (p e)")

                # ---- x[g]: aug rows [x(64); 1], b = p*8+e chunking
                xa = ld.tile([128, E, D], F32, tag="xa")
                dma(out=xa, in_=x_in[g].rearrange("(p e) d -> p e d", p=128))
                xsq = ld.tile([128, E, D], F32, tag="xsq")
                nc.vector.tensor_mul(xsq, xa, xa)
                x2 = ld.tile([128, E], F32, tag="x2")
                nc.vector.reduce_sum(x2, xsq, axis=mybir.AxisListType.X)
                xf = opp.tile([128, E, 65], F16, tag="xf")
                nc.vector.tensor_copy(xf[:, :, 0:64], xa)
                nc.vector.memset(xf[:, :, 64], 1.0)
                psx = pt.tile([65, E, 128], F16, tag="psT")
                for e in range(E):
                    nc.tensor.transpose(psx[:, e], xf[:, e], ident)
                lhsT = opp.tile([65, E, 128], F16, tag="lhsT")
                nc.vector.tensor_copy(lhsT, psx)

                # ---- d2 blocks; sqrt(d2 + x2[b]) via ACT bias; fp16 store
                o_big = obp.tile([128, E, K], F16, tag="o_big")
                for e in range(E):
                    psum = ps.tile([128, K], F32, tag="psum")
                    for u in range(2):
                        nc.tensor.matmul(
                            psum[:, u * 512:(u + 1) * 512],
                            lhsT=lhsT[:, e],
                            rhs=cTf[:, u * 512:(u + 1) * 512],
                            start=True, stop=True)
                    nc.scalar.activation(
                        out=o_big[:, e], in_=psum, func=SQRT,
                        bias=x2[:, e:e + 1], scale=1.0)
                oh = out[g].rearrange("(p f e) k -> p f (e k)", p=128, f=2)
                dma(out=oh[:, 0], in_=o_big[:, 0:4])
                dma(out=oh[:, 1], in_=o_big[:, 4:8])

            def body():
                for g in range(G):
                    group_body(g)

            if repeat == 1:
                body()
            elif repeat % 4 == 0:
                with tc.For_i(0, repeat // 4, 1,
                              hint_engines=(mybir.EngineType.PE,)):
                    for _ in range(4):
                        body()
            elif repeat % 2 == 0:
                with tc.For_i(0, repeat // 2, 1,
                              hint_engines=(mybir.EngineType.PE,)):
                    body()
                    body()
            else:
                with tc.For_i(0, repeat, 1,
                              hint_engines=(mybir.EngineType.PE,)):
                    body()
    nc.compile()
    return nc


def get_nc(repeat: int = 1):
    if repeat not in _cache:
        _cache[repeat] = build_nc(repeat)
    return _cache[repeat]


def run(x: np.ndarray, centroids: np.ndarray, repeat: int = 1):
    nc = get_nc(repeat)
    x = np.ascontiguousarray(x, dtype=np.float32)
    centroids = np.ascontiguousarray(centroids, dtype=np.float32)
    in_maps = [
        {"x": x[:, c * BS:(c + 1) * BS, :], "centroids": centroids}
        for c in range(N_CORES)
    ]
    res = run_bass_kernel_spmd(nc, in_maps, list(range(N_CORES)))
    full = np.empty((G, B, K), dtype=np.float32)
    for c in range(N_CORES):
        full[:, c * BS:(c + 1) * BS, :] = res.results[c]["out"]
    return full


def kernel(x: np.ndarray, centroids: np.ndarray) -> np.ndarray:
    return run(x, centroids, repeat=1)
